# revision 1
# baseline (speedup 1.0000x reference)
"""Trainium2 Bass kernel for DepthwiseXCorrAug.

Computes, for B=64 samples sharded 8-per-core across 8 NeuronCores:
  k = relu(bn(conv3x3_valid(kernel_in, w_k)))     # [B,256,5,5]
  s = relu(bn(conv3x3_same(search_in, w_s)))      # [B,256,31,31]
  out = per-sample per-channel xcorr(s, k), pad 2 # [B,256,31,31]

Device strategy (per core):
  - conv branches as float32r (TF32-like, full PE rate) matmuls over
    (ci-block x 3x3-tap) accumulated in PSUM; BN folded into weights on
    host, bias+ReLU applied by ScalarE on PSUM eviction.
  - depthwise xcorr as bf16 diagonal-weight matmuls: 16 concurrent 32x32
    PE tiles (4 channel-blocks x 4 samples) accumulate the 25 taps in PSUM.
"""

import sys
import types

sys.path.insert(0, "/opt/trn_rl_repo")

import numpy as np

import concourse.bass as bass
import concourse.mybir as mybir
import concourse.tile as tile
from concourse import bacc
from concourse.bass_utils import run_bass_kernel_spmd

EPS = 1e-5
N_CORES = 8
B, CIN, HID = 64, 256, 256
SPC = B // N_CORES  # samples per core

_cached_nc = None
last_results = None  # set by kernel(); used by test harness for profiling


def _round_fp32r(a: np.ndarray) -> np.ndarray:
    """Round fp32 to the PE's FP32R format (8-bit exp, 11-bit mantissa), RNE."""
    b = a.view(np.uint32).astype(np.uint64)
    lsb = (b >> 12) & 1
    r = b + 0x7FF + lsb
    return (r & ~np.uint64(0xFFF)).astype(np.uint32).view(np.float32)


def _build_program():
    f32 = mybir.dt.float32
    f32r = mybir.dt.float32r
    bf16 = mybir.dt.bfloat16
    RELU = mybir.ActivationFunctionType.Relu

    nc = bacc.Bacc("TRN2", target_bir_lowering=False, debug=False,
                   num_devices=N_CORES)

    wTs_d = [nc.dram_tensor(f"wTs{cb}", [128, 2304], f32r, kind="ExternalInput").ap()
             for cb in range(2)]
    wTk_d = [nc.dram_tensor(f"wTk{cb}", [128, 2304], f32r, kind="ExternalInput").ap()
             for cb in range(2)]
    xk_d = [nc.dram_tensor(f"xk{cb}", [128, 2304], f32r, kind="ExternalInput").ap()
            for cb in range(2)]
    xs_d = nc.dram_tensor("xs", [SPC, 2, 128, 33 * 34], f32r, kind="ExternalInput").ap()
    bk_d = nc.dram_tensor("bk", [2, 128, 1], f32, kind="ExternalInput").ap()
    bs_d = nc.dram_tensor("bs", [2, 128, 1], f32, kind="ExternalInput").ap()
    m32_d = nc.dram_tensor("m32", [128, 32], bf16, kind="ExternalInput").ap()
    out_d = nc.dram_tensor("out", [SPC, CIN, 31, 31], f32, kind="ExternalOutput").ap()
    out_flat = out_d.rearrange("s c h w -> s c (h w)")

    with tile.TileContext(nc) as tc:
        with tc.tile_pool(name="wp", bufs=1) as wp, \
             tc.tile_pool(name="spin", bufs=8) as spin_pool, \
             tc.tile_pool(name="spoutp", bufs=1) as spout_pool, \
             tc.tile_pool(name="stripp", bufs=1) as strip_pool, \
             tc.tile_pool(name="xop", bufs=8) as xout_pool, \
             tc.tile_pool(name="psc", bufs=4, space="PSUM") as psc, \
             tc.tile_pool(name="psx", bufs=4, space="PSUM") as psx_pool:

            # ---- persistent inputs ----
            wTs = [wp.tile([128, 2304], f32r, tag=f"wTs{cb}", name=f"wTs{cb}")
                   for cb in range(2)]
            wTk = [wp.tile([128, 2304], f32r, tag=f"wTk{cb}", name=f"wTk{cb}")
                   for cb in range(2)]
            xk = [wp.tile([128, 2304], f32r, tag=f"xk{cb}", name=f"xk{cb}")
                  for cb in range(2)]
            bk = [wp.tile([128, 1], f32, tag=f"bk{ob}", name=f"bk{ob}")
                  for ob in range(2)]
            bs = [wp.tile([128, 1], f32, tag=f"bs{ob}", name=f"bs{ob}")
                  for ob in range(2)]
            m32 = wp.tile([128, 32], bf16, tag="m32", name="m32")
            kf = [wp.tile([128, 200], f32, tag=f"kf{ob}", name=f"kf{ob}")
                  for ob in range(2)]

            # spin prefetch state (filled by prefetch_pair below)
            spin_views = {}

            def prefetch_pair(pair):
                s0 = pair * 2
                for s in (s0, s0 + 1):
                    for cb in range(2):
                        t_in = spin_pool.tile([128, 33 * 34], f32r,
                                              tag="spin", name=f"spin{s}_{cb}")
                        nc.sync.dma_start(t_in[:], xs_d[s, cb])
                        spin_views[(s, cb)] = t_in[:].rearrange(
                            "p (h w) -> p h w", h=33, w=34)

            for ob in range(2):
                nc.sync.dma_start(bk[ob][:], bk_d[ob])
                nc.sync.dma_start(bs[ob][:], bs_d[ob])
            nc.sync.dma_start(m32[:], m32_d)
            prefetch_pair(0)
            # big loads split into column chunks so no single DMA queue
            # serializes the critical path; conv_k inputs (wTk/xk) before the
            # pair-1 search tiles, which aren't needed until later
            CH = 576
            for cb in range(2):
                for c0 in range(0, 2304, CH):
                    nc.sync.dma_start(wTs[cb][:, c0:c0 + CH],
                                      wTs_d[cb][:, c0:c0 + CH])
            for cb in range(2):
                for c0 in range(0, 2304, CH):
                    nc.sync.dma_start(wTk[cb][:, c0:c0 + CH],
                                      wTk_d[cb][:, c0:c0 + CH])
                for c0 in range(0, 2304, CH):
                    nc.sync.dma_start(xk[cb][:, c0:c0 + CH],
                                      xk_d[cb][:, c0:c0 + CH])
            prefetch_pair(1)

            # ---- conv_k: all 8 samples batched on the free dim (N=256) ----
            def emit_conv_k():
                for ob in range(2):
                    pk = psc.tile([128, 512], f32, tag="conv", name=f"pk{ob}")
                    idx = 0
                    for cb in range(2):
                        for t in range(9):
                            nc.tensor.matmul(
                                pk[:, 0:256],
                                wTs_lhs(wTk, cb, t, ob),
                                xk[cb][:, t * 256:(t + 1) * 256],
                                start=(idx == 0), stop=(idx == 17))
                            idx += 1
                    nc.scalar.activation(kf[ob][:], pk[:, 0:200], RELU,
                                         bias=bk[ob][:, 0:1], scale=1.0)

            # ---- strips: bf16 diagonal weights for the xcorr ----
            strips = {}
            for s in range(SPC):
                for ob in range(2):
                    strips[(s, ob)] = strip_pool.tile(
                        [128, 800], bf16,
                        tag=f"strip{s}_{ob}", name=f"strip{s}_{ob}")

            def emit_strips():
                for ob in range(2):
                    for s in range(SPC):
                        st = strips[(s, ob)]
                        for t in range(25):
                            nc.vector.tensor_scalar(
                                st[:, t * 32:(t + 1) * 32], m32[:],
                                kf[ob][:, s * 25 + t:s * 25 + t + 1],
                                None, mybir.AluOpType.mult)

            # ---- spout tiles (bf16, zero borders) ----
            spout = {}
            for s in range(SPC):
                for ob in range(2):
                    sp = spout_pool.tile([128, 35 * 35], bf16,
                                         tag=f"spout{s}_{ob}", name=f"spout{s}_{ob}")
                    spout[(s, ob)] = sp
                    nc.gpsimd.memset(sp[:], 0.0)

            # ---- main: conv pairs interleaved with xcorr chunks ----
            def conv_s_pair(pair):
                s0 = pair * 2
                views = spin_views
                for ob in range(2):
                    ptiles = {}
                    for s in (s0, s0 + 1):
                        for ci, (y0, nr) in enumerate([(0, 16), (16, 15)]):
                            ptiles[(s, ci)] = psc.tile(
                                [128, 512], f32, tag="conv",
                                name=f"pc{s}_{ob}_{ci}")
                    idx = 0
                    for cb in range(2):
                        for t in range(9):
                            dy, dx = t // 3, t % 3
                            lhsT = wTs[cb][:, (t * 2 + ob) * 128:(t * 2 + ob + 1) * 128]
                            for s in (s0, s0 + 1):
                                for ci, (y0, nr) in enumerate([(0, 16), (16, 15)]):
                                    nc.tensor.matmul(
                                        ptiles[(s, ci)][:, 0:nr * 32],
                                        lhsT,
                                        views[(s, cb)][:, y0 + dy:y0 + dy + nr,
                                                       dx:dx + 32],
                                        start=(idx == 0), stop=(idx == 17))
                            idx += 1
                    for s in (s0, s0 + 1):
                        sov = spout[(s, ob)][:].rearrange(
                            "p (h w) -> p h w", h=35, w=35)
                        for ci, (y0, nr) in enumerate([(0, 16), (16, 15)]):
                            pv = ptiles[(s, ci)][:, 0:nr * 32].rearrange(
                                "p (h w) -> p h w", h=nr, w=32)
                            nc.scalar.activation(
                                sov[:, 2 + y0:2 + y0 + nr, 2:33],
                                pv[:, :, 0:31], RELU,
                                bias=bs[ob][:, 0:1], scale=1.0)

            def xcorr_chunk(g, ob, ci, pool=None, tag="xc"):
                    pool = pool or psx_pool
                    sovs = [spout[(g * 4 + j, ob)][:].rearrange(
                        "p (h w) -> p h w", h=35, w=35) for j in range(4)]
                    for (y0, nr) in [[(0, 16), (16, 15)][ci]]:
                        N = nr * 31
                        px = [pool.tile([128, 512], f32, tag=tag,
                                        name=f"px{g}_{ob}_{y0}_{i}")
                              for i in range(4)]
                        for t in range(25):
                            dy, dx = t // 5, t % 5
                            for i in range(4):
                                for j in range(4):
                                    st = strips[(g * 4 + j, ob)]
                                    nc.tensor.matmul(
                                        px[i][32 * j:32 * j + 32, 0:N],
                                        st[32 * i:32 * i + 32, t * 32:(t + 1) * 32],
                                        sovs[j][32 * i:32 * i + 32,
                                                y0 + dy:y0 + dy + nr, dx:dx + 31],
                                        start=(t == 0), stop=(t == 24),
                                        tile_position=(32 * i, 32 * j))
                        for i in range(4):
                            xo = xout_pool.tile([128, 496], f32, tag="xo",
                                                name=f"xo{g}_{ob}_{y0}_{i}")
                            if i % 2 == 0:
                                nc.vector.tensor_copy(xo[:, 0:N], px[i][:, 0:N])
                            else:
                                nc.scalar.copy(xo[:, 0:N], px[i][:, 0:N])
                            dst = out_flat[g * 4:g * 4 + 4,
                                           ob * 128 + 32 * i:ob * 128 + 32 * i + 32,
                                           y0 * 31:y0 * 31 + N]
                            nc.sync.dma_start(dst, xo[:, 0:N])

            conv_s_pair(0)
            prefetch_pair(2)
            emit_conv_k()
            emit_strips()
            conv_s_pair(1)
            prefetch_pair(3)
            xcorr_chunk(0, 0, 0)
            conv_s_pair(2)
            xcorr_chunk(0, 0, 1)
            conv_s_pair(3)
            for n, args in enumerate([(0, 1, 0), (0, 1, 1), (1, 0, 0),
                                      (1, 0, 1), (1, 1, 0), (1, 1, 1)]):
                if n % 2 == 0:
                    xcorr_chunk(*args)
                else:
                    xcorr_chunk(*args, pool=psc, tag="conv")

    nc.compile()
    return nc


def wTs_lhs(w, cb, t, ob):
    return w[cb][:, (t * 2 + ob) * 128:(t * 2 + ob + 1) * 128]


def _host_prep(kernel, search, w_k, g_k, b_k, m_k, v_k, w_s, g_s, b_s, m_s, v_s):
    import ml_dtypes

    def fold(w, g, b, m, v):
        scale = g / np.sqrt(v + EPS)
        return (w * scale[:, None, None, None]).astype(np.float32), \
               (b - m * scale).astype(np.float32)

    wkf, bias_k = fold(w_k, g_k, b_k, m_k, v_k)
    wsf, bias_s = fold(w_s, g_s, b_s, m_s, v_s)

    def packT(w):  # [o, ci, 3, 3] -> [cb, ci, (t, ob, o)] fp32r
        arr = w.reshape(2, 128, 2, 128, 9).transpose(2, 3, 4, 0, 1)
        return _round_fp32r(np.ascontiguousarray(arr, dtype=np.float32)
                            ).reshape(2, 128, 2304)

    wTk = packT(wkf)
    wTs = packT(wsf)

    M32 = np.zeros((128, 32), dtype=np.float32)
    for p in range(128):
        M32[p, p % 32] = 1.0
    M32 = M32.astype(ml_dtypes.bfloat16)

    bk = np.ascontiguousarray(bias_k.reshape(2, 128, 1))
    bs = np.ascontiguousarray(bias_s.reshape(2, 128, 1))

    in_maps = []
    for core in range(N_CORES):
        kin = kernel[core * SPC:(core + 1) * SPC]
        sin = search[core * SPC:(core + 1) * SPC]

        Xk = np.zeros((2, 128, 9, 256), dtype=np.float32)
        for t in range(9):
            dy, dx = t // 3, t % 3
            p = kin[:, :, dy:dy + 5, dx:dx + 5].reshape(SPC, 2, 128, 25)
            Xk[:, :, t, :200] = p.transpose(1, 2, 0, 3).reshape(2, 128, 200)
        Xk = _round_fp32r(Xk).reshape(2, 128, 2304)

        Xs = np.zeros((SPC, 2, 128, 33, 34), dtype=np.float32)
        Xs[:, :, :, 1:32, 1:32] = sin.reshape(SPC, 2, 128, 31, 31)
        Xs = _round_fp32r(Xs).reshape(SPC, 2, 128, 33 * 34)

        in_maps.append({
            "wTs0": wTs[0], "wTs1": wTs[1],
            "wTk0": wTk[0], "wTk1": wTk[1],
            "xk0": Xk[0], "xk1": Xk[1],
            "xs": Xs, "bk": bk, "bs": bs, "m32": M32,
        })
    return in_maps


def kernel(kernel, search, w_k, g_k, b_k, m_k, v_k, w_s, g_s, b_s, m_s, v_s,
           _trace=False):
    global _cached_nc, last_results
    args = [np.ascontiguousarray(np.asarray(x, dtype=np.float32)) for x in
            (kernel, search, w_k, g_k, b_k, m_k, v_k, w_s, g_s, b_s, m_s, v_s)]
    if _cached_nc is None:
        _cached_nc = _build_program()
    nc = _cached_nc
    in_maps = _host_prep(*args)
    res = run_bass_kernel_spmd(nc, in_maps, core_ids=list(range(N_CORES)),
                               trace=_trace)
    last_results = res
    out = np.concatenate([res.results[i]["out"] for i in range(N_CORES)], axis=0)
    return np.ascontiguousarray(out.astype(np.float32))



# revision 3
# speedup vs baseline: 1.0591x; 1.0591x over previous
"""Trainium2 Bass kernel for DepthwiseXCorrAug.

Computes, for B=64 samples sharded 8-per-core across 8 NeuronCores:
  k = relu(bn(conv3x3_valid(kernel_in, w_k)))     # [B,256,5,5]
  s = relu(bn(conv3x3_same(search_in, w_s)))      # [B,256,31,31]
  out = per-sample per-channel xcorr(s, k), pad 2 # [B,256,31,31]

Device strategy (per core):
  - conv branches as float32r matmuls over (ci-block x 3x3-tap) accumulated
    in PSUM; BN folded into weights on host, bias+ReLU by ScalarE on PSUM
    eviction into zero-bordered bf16 spout tiles.
  - depthwise xcorr as bf16 diagonal-weight matmuls: 16 concurrent 32x32
    PE tiles (4 channel-blocks x 4 samples) accumulate the 25 taps in PSUM.
  - DMA: partition-major packed layouts so every descriptor is a multi-KB
    row; head-critical loads split into 32-partition groups across both
    HWDGE rings; outputs land in a packed [g,ob,i,128,961] layout
    (transposed back on host) so the drain never throttles the PE.
"""

import sys

sys.path.insert(0, "/opt/trn_rl_repo")

import numpy as np

import concourse.bass as bass
import concourse.mybir as mybir
import concourse.tile as tile
from concourse import bacc
from concourse.bass_utils import run_bass_kernel_spmd

EPS = 1e-5
N_CORES = 8
B, CIN, HID = 64, 256, 256
SPC = B // N_CORES  # samples per core

_cached_nc = None
last_results = None  # set by kernel(); used by test harness for profiling


def _round_fp32r(a: np.ndarray) -> np.ndarray:
    """Round fp32 to the PE's FP32R format (8-bit exp, 11-bit mantissa), RNE."""
    b = a.view(np.uint32).astype(np.uint64)
    lsb = (b >> 12) & 1
    r = b + 0x7FF + lsb
    return (r & ~np.uint64(0xFFF)).astype(np.uint32).view(np.float32)


def _build_program():
    f32 = mybir.dt.float32
    f32r = mybir.dt.float32r
    bf16 = mybir.dt.bfloat16
    RELU = mybir.ActivationFunctionType.Relu
    MULT = mybir.AluOpType.mult

    nc = bacc.Bacc("TRN2", target_bir_lowering=False, debug=False,
                   num_devices=N_CORES)

    wTs_d = nc.dram_tensor("wTs", [128, 4608], f32r, kind="ExternalInput").ap()
    wTk_d = nc.dram_tensor("wTk", [128, 4608], f32r, kind="ExternalInput").ap()
    xk_d = nc.dram_tensor("xk", [128, 4608], f32r, kind="ExternalInput").ap()
    xs_d = nc.dram_tensor("xs", [SPC, 128, 2244], f32r, kind="ExternalInput").ap()
    cst_d = nc.dram_tensor("cst", [128, 4], f32, kind="ExternalInput").ap()
    m32_d = nc.dram_tensor("m32", [128, 32], bf16, kind="ExternalInput").ap()
    outp_d = nc.dram_tensor("outp", [2, 2, 4, 128, 961], f32,
                            kind="ExternalOutput").ap()

    with tile.TileContext(nc) as tc:
        with tc.tile_pool(name="wp", bufs=1) as wp, \
             tc.tile_pool(name="spin", bufs=5) as spin_pool, \
             tc.tile_pool(name="xop", bufs=8) as xout_pool, \
             tc.tile_pool(name="psc", bufs=4, space="PSUM") as psc, \
             tc.tile_pool(name="psx", bufs=4, space="PSUM") as psx:

            # ---- persistent tiles ----
            wTs = wp.tile([128, 4608], f32r, tag="wTs", name="wTs")
            wTk = wp.tile([128, 4608], f32r, tag="wTk", name="wTk")
            xk = wp.tile([128, 4608], f32r, tag="xk", name="xk")
            cst = wp.tile([128, 4], f32, tag="cst", name="cst")
            m32 = wp.tile([128, 32], bf16, tag="m32", name="m32")
            kf = [wp.tile([128, 200], f32, tag=f"kf{ob}", name=f"kf{ob}")
                  for ob in range(2)]
            spout = {}
            strips = {}
            for s in range(SPC):
                for ob in range(2):
                    spout[(s, ob)] = wp.tile(
                        [128, 35 * 35], bf16, tag=f"sp{s}_{ob}", name=f"sp{s}_{ob}")
                    strips[(s, ob)] = wp.tile(
                        [128, 800], bf16, tag=f"st{s}_{ob}", name=f"st{s}_{ob}")

            # zero spout borders while the first DMAs are in flight
            for s in range(SPC):
                for ob in range(2):
                    nc.gpsimd.memset(spout[(s, ob)][:], 0.0)

            # ---- input DMA: head-critical first, split across both rings ----
            spin = {}

            def alloc_spin(s):
                spin[s] = spin_pool.tile([128, 2244], f32r, tag="spin",
                                         name=f"spin{s}")

            alloc_spin(0)
            for q in range(4):
                nc.sync.dma_start(wTs[32 * q:32 * q + 32, :],
                                  wTs_d[32 * q:32 * q + 32, :])
            for q in range(4):
                nc.scalar.dma_start(spin[0][32 * q:32 * q + 32, :],
                                    xs_d[0, 32 * q:32 * q + 32, :])
            nc.sync.dma_start(cst[:], cst_d)
            nc.sync.dma_start(m32[:], m32_d)
            alloc_spin(1)
            nc.sync.dma_start(spin[1][:], xs_d[1])
            nc.sync.dma_start(wTk[:], wTk_d)
            nc.sync.dma_start(xk[:], xk_d)
            for s in range(2, SPC):
                alloc_spin(s)
                nc.sync.dma_start(spin[s][:], xs_d[s])

            def spin_view(s, cb):
                return spin[s][:, cb * 1122:(cb + 1) * 1122].rearrange(
                    "p (h w) -> p h w", h=33, w=34)

            # ---- conv_s for one sample ----
            def conv_s_sample(s):
                for ob in range(2):
                    p0 = psc.tile([128, 512], f32, tag="conv", name=f"c{s}{ob}0")
                    p1 = psc.tile([128, 512], f32, tag="conv", name=f"c{s}{ob}1")
                    idx = 0
                    for cb in range(2):
                        v = spin_view(s, cb)
                        for t in range(9):
                            dy, dx = t // 3, t % 3
                            lhsT = wTs[:, cb * 2304 + (t * 2 + ob) * 128:
                                       cb * 2304 + (t * 2 + ob + 1) * 128]
                            nc.tensor.matmul(
                                p0[:, 0:512], lhsT,
                                v[:, dy:dy + 16, dx:dx + 32],
                                start=(idx == 0), stop=(idx == 17))
                            nc.tensor.matmul(
                                p1[:, 0:480], lhsT,
                                v[:, 16 + dy:16 + dy + 15, dx:dx + 32],
                                start=(idx == 0), stop=(idx == 17))
                            idx += 1
                    sov = spout[(s, ob)][:].rearrange(
                        "p (h w) -> p h w", h=35, w=35)
                    nc.scalar.activation(
                        sov[:, 2:18, 2:33],
                        p0[:, 0:512].rearrange(
                            "p (h w) -> p h w", h=16, w=32)[:, :, 0:31],
                        RELU, bias=cst[:, 2 + ob:3 + ob], scale=1.0)
                    nc.scalar.activation(
                        sov[:, 18:33, 2:33],
                        p1[:, 0:480].rearrange(
                            "p (h w) -> p h w", h=15, w=32)[:, :, 0:31],
                        RELU, bias=cst[:, 2 + ob:3 + ob], scale=1.0)

            # ---- conv_k: all 8 samples batched on the free dim ----
            def emit_conv_k():
                for ob in range(2):
                    pk = psc.tile([128, 512], f32, tag="conv", name=f"pk{ob}")
                    idx = 0
                    for cb in range(2):
                        for t in range(9):
                            nc.tensor.matmul(
                                pk[:, 0:256],
                                wTk[:, cb * 2304 + (t * 2 + ob) * 128:
                                    cb * 2304 + (t * 2 + ob + 1) * 128],
                                xk[:, cb * 2304 + t * 256:
                                   cb * 2304 + t * 256 + 256],
                                start=(idx == 0), stop=(idx == 17))
                            idx += 1
                    nc.scalar.activation(kf[ob][:], pk[:, 0:200], RELU,
                                         bias=cst[:, ob:ob + 1], scale=1.0)

            # ---- strips: bf16 diagonal weights, spread over Vector+GpSimd ----
            def emit_strips(g):
                n = 0
                for ob in range(2):
                    for s in range(g * 4, g * 4 + 4):
                        st = strips[(s, ob)]
                        for t in range(25):
                            eng = nc.vector if n % 2 == 0 else nc.gpsimd
                            eng.tensor_scalar(
                                st[:, t * 32:(t + 1) * 32], m32[:],
                                kf[ob][:, s * 25 + t:s * 25 + t + 1],
                                None, MULT)
                            n += 1

            # ---- xcorr for 4 samples x 1 ob-half (both row chunks) ----
            def xcorr_chunk(g, ob):
                xos = [xout_pool.tile([128, 961], f32, tag="xo",
                                      name=f"xo{g}{ob}{i}") for i in range(4)]
                sovs = [spout[(g * 4 + j, ob)][:].rearrange(
                    "p (h w) -> p h w", h=35, w=35) for j in range(4)]
                for ci, (y0, nr, pool, ptag) in enumerate(
                        [(0, 16, psx, "xc"), (16, 15, psc, "conv")]):
                    N = nr * 31
                    px = [pool.tile([128, 512], f32, tag=ptag,
                                    name=f"px{g}{ob}{ci}{i}") for i in range(4)]
                    for t in range(25):
                        dy, dx = t // 5, t % 5
                        for i in range(4):
                            for j in range(4):
                                st = strips[(g * 4 + j, ob)]
                                nc.tensor.matmul(
                                    px[i][32 * j:32 * j + 32, 0:N],
                                    st[32 * i:32 * i + 32, t * 32:(t + 1) * 32],
                                    sovs[j][32 * i:32 * i + 32,
                                            y0 + dy:y0 + dy + nr, dx:dx + 31],
                                    start=(t == 0), stop=(t == 24),
                                    tile_position=(32 * i, 32 * j))
                    for i in range(4):
                        if i % 2 == 0:
                            nc.vector.tensor_copy(
                                xos[i][:, y0 * 31:y0 * 31 + N], px[i][:, 0:N])
                        else:
                            nc.scalar.copy(
                                xos[i][:, y0 * 31:y0 * 31 + N], px[i][:, 0:N])
                for i in range(4):
                    for q in range(4):
                        eng = nc.sync if i < 2 else nc.scalar
                        eng.dma_start(
                            outp_d[g, ob, i, 32 * q:32 * q + 32, :],
                            xos[i][32 * q:32 * q + 32, :])

            # ---- main schedule ----
            conv_s_sample(0)
            conv_s_sample(1)
            emit_conv_k()
            emit_strips(0)
            conv_s_sample(2)
            conv_s_sample(3)
            xcorr_chunk(0, 0)
            conv_s_sample(4)
            xcorr_chunk(0, 1)
            conv_s_sample(5)
            emit_strips(1)
            conv_s_sample(6)
            conv_s_sample(7)
            xcorr_chunk(1, 0)
            xcorr_chunk(1, 1)

    nc.compile()
    return nc


def _host_prep(kernel, search, w_k, g_k, b_k, m_k, v_k, w_s, g_s, b_s, m_s, v_s):
    import ml_dtypes

    def fold(w, g, b, m, v):
        scale = g / np.sqrt(v + EPS)
        return (w * scale[:, None, None, None]).astype(np.float32), \
               (b - m * scale).astype(np.float32)

    wkf, bias_k = fold(w_k, g_k, b_k, m_k, v_k)
    wsf, bias_s = fold(w_s, g_s, b_s, m_s, v_s)

    def packT(w):  # [o, ci, 3, 3] -> [ci(128), cb*2304 + (t,ob,o)] fp32r
        arr = w.reshape(2, 128, 2, 128, 9).transpose(2, 3, 4, 0, 1)
        arr = _round_fp32r(np.ascontiguousarray(arr, dtype=np.float32)
                           ).reshape(2, 128, 2304)
        return np.ascontiguousarray(np.concatenate([arr[0], arr[1]], axis=1))

    wTk = packT(wkf)
    wTs = packT(wsf)

    M32 = np.zeros((128, 32), dtype=np.float32)
    for p in range(128):
        M32[p, p % 32] = 1.0
    M32 = M32.astype(ml_dtypes.bfloat16)

    cst = np.ascontiguousarray(
        np.stack([bias_k[0:128], bias_k[128:256],
                  bias_s[0:128], bias_s[128:256]], axis=1).astype(np.float32))

    in_maps = []
    for core in range(N_CORES):
        kin = kernel[core * SPC:(core + 1) * SPC]
        sin = search[core * SPC:(core + 1) * SPC]

        Xk = np.zeros((2, 128, 9, 256), dtype=np.float32)
        for t in range(9):
            dy, dx = t // 3, t % 3
            p = kin[:, :, dy:dy + 5, dx:dx + 5].reshape(SPC, 2, 128, 25)
            Xk[:, :, t, :200] = p.transpose(1, 2, 0, 3).reshape(2, 128, 200)
        Xk = _round_fp32r(Xk).reshape(2, 128, 2304)
        Xk = np.ascontiguousarray(np.concatenate([Xk[0], Xk[1]], axis=1))

        Xs = np.zeros((SPC, 2, 128, 33, 34), dtype=np.float32)
        Xs[:, :, :, 1:32, 1:32] = sin.reshape(SPC, 2, 128, 31, 31)
        Xs = _round_fp32r(Xs).transpose(0, 2, 1, 3, 4)
        Xs = np.ascontiguousarray(Xs.reshape(SPC, 128, 2244))

        in_maps.append({
            "wTs": wTs, "wTk": wTk, "xk": Xk, "xs": Xs,
            "cst": cst, "m32": M32,
        })
    return in_maps


def kernel(kernel, search, w_k, g_k, b_k, m_k, v_k, w_s, g_s, b_s, m_s, v_s,
           _trace=False):
    global _cached_nc, last_results
    args = [np.ascontiguousarray(np.asarray(x, dtype=np.float32)) for x in
            (kernel, search, w_k, g_k, b_k, m_k, v_k, w_s, g_s, b_s, m_s, v_s)]
    if _cached_nc is None:
        _cached_nc = _build_program()
    nc = _cached_nc
    in_maps = _host_prep(*args)
    res = run_bass_kernel_spmd(nc, in_maps, core_ids=list(range(N_CORES)),
                               trace=_trace)
    last_results = res
    outs = []
    for i in range(N_CORES):
        op = res.results[i]["outp"]  # [2, 2, 4, 128, 961]
        op = op.reshape(2, 2, 4, 4, 32, 961).transpose(0, 3, 1, 2, 4, 5)
        outs.append(op.reshape(SPC, CIN, 31, 31))
    out = np.concatenate(outs, axis=0)
    return np.ascontiguousarray(out.astype(np.float32))
